# revision 11
# baseline (speedup 1.0000x reference)
"""Trainium2 Bass kernel for nn_MLP_4337916970028.

Computes: out = gelu(x @ up) @ down^T where
  up   = S @ fwht(sign * w_up, 1/sqrt(N)).T      [1024, 4096]
  down = S @ fwht(sign * w_down.T, 1/sqrt(N)).T  [1024, 4096]
with S the [1024, 8192] one-nonzero-per-column JL projection.

Device algebra per core (hidden dim sharded 512/core for the weight prep,
tokens sharded 2048/core for the main matmuls):
  G = scale * H_8192 @ diag(sign) @ w_slice          [8192, 512]
computed as two factored PE stages (H_8192 = H_64 (x) H_128):
  stage 1 contracts the minor 128 block with per-chunk lhsT tiles
  (sign*scale folded in on host), streamed back to DRAM contiguously;
  stage 2 contracts the major 64 block with block-diagonal pair lhsT
  tiles (projection values v_c folded in on host).
The sparse projection S is then applied with dma_gather (rows of G
gathered per output row, padded per 128-row batch to that batch's max
multiplicity) + a DVE tree-sum.  Only two 1 MB-per-core AllGathers
(up, down^T) remain.  Mains run token-blocked with h resident in SBUF.
"""
import math
import os
import sys
import types

sys.path.insert(0, "/opt/trn_rl_repo")
import numpy as np  # noqa: E402

import concourse.bass as bass  # noqa: E402
import concourse.mybir as mybir  # noqa: E402
import concourse.tile as tile  # noqa: E402
from concourse import bacc  # noqa: E402
from concourse.bass_utils import run_bass_kernel_spmd  # noqa: E402
from concourse.masks import make_identity  # noqa: E402

F32 = mybir.dt.float32
F16 = mybir.dt.float16
I16 = mybir.dt.int16
AF = mybir.ActivationFunctionType

NC = 8
R = 1024      # n_embd
C = 8192      # hadamard dim N
D = 4096      # hidden 4*n_embd
T = 16384     # tokens
DS = D // NC  # 512 hidden per core (weight-prep shard)
TS = T // NC  # 2048 tokens per core (main shard)
TQ = 512      # main-phase token block
SCALE = 1.0 / math.sqrt(C)
ZROW = C      # zero row index in gd tensors

_NC_CACHE = {}
last_exec_time_ns = None


def _register_ntff_hook():
    try:
        import antenv.axon_hooks  # noqa: F401
        return
    except ImportError:
        pass
    try:
        from trn_agent_boot.trn_boot import _ntff_profile_via_ctypes
        hook = _ntff_profile_via_ctypes("/opt/axon/libaxon_pjrt.so")
    except Exception:
        return
    mod = types.ModuleType("antenv.axon_hooks")
    mod._hook = hook
    mod.get_axon_ntff_profile_hook = lambda: mod._hook
    mod.set_axon_ntff_profile_hook = lambda h: setattr(mod, "_hook", h)
    sys.modules["antenv.axon_hooks"] = mod
    import antenv
    antenv.axon_hooks = mod


def _hadamard(n):
    H = np.array([[1.0]], dtype=np.float64)
    while H.shape[0] < n:
        H = np.block([[H, H], [H, -H]])
    return H


def _tree_sum(nc, tview, nchunks, out_ap):
    """Sum `nchunks` [128, 512] chunks of tview [128, n, 512] into out_ap.

    In-place pairwise halving on the free axis; the final add (or copy)
    lands in out_ap.
    """
    n = nchunks
    if n == 1:
        nc.vector.tensor_copy(out_ap, tview[:, 0, :])
        return
    while n > 2:
        if n % 2 == 1:
            nc.vector.tensor_add(tview[:, 0, :], tview[:, 0, :],
                                 tview[:, n - 1, :])
            n -= 1
        if n == 2:
            break
        m = n // 2
        nc.vector.tensor_add(
            tview[:, 0:m, :].rearrange("p a d -> p (a d)"),
            tview[:, 0:m, :].rearrange("p a d -> p (a d)"),
            tview[:, m:n, :].rearrange("p a d -> p (a d)"))
        n = m
    nc.vector.tensor_add(out_ap, tview[:, 0, :], tview[:, 1, :])


def _build(pads):
    """pads: tuple of 8 per-batch slot counts (max multiplicity per
    128-row block of the projection)."""
    smax = max(pads)
    gidx_cols = sum(128 * p // 16 for p in pads)

    nc = bacc.Bacc("TRN2", target_bir_lowering=False, debug=False, num_devices=NC)
    xt_in = nc.dram_tensor("xt_in", [R, TS], F16, kind="ExternalInput").ap()
    wup_in = nc.dram_tensor("wup_in", [C, DS], F16, kind="ExternalInput").ap()
    wdn_in = nc.dram_tensor("wdn_in", [C, DS], F16, kind="ExternalInput").ap()
    lh1_in = nc.dram_tensor("lh1_in", [128, 64 * 128], F16,
                            kind="ExternalInput").ap()
    lh2_in = nc.dram_tensor("lh2_in", [128, 64 * 128], F16,
                            kind="ExternalInput").ap()
    gidx_in = nc.dram_tensor("gidx_in", [128, gidx_cols], I16,
                             kind="ExternalInput").ap()
    out_ext = nc.dram_tensor("out", [TS, R], F32, kind="ExternalOutput").ap()
    debug = bool(os.environ.get("KERNEL_DEBUG"))
    if debug:
        dbg_up = nc.dram_tensor("dbg_up", [NC * R, DS // 2], F32,
                                kind="ExternalOutput").ap()
        dbg_dn = nc.dram_tensor("dbg_dn", [D, R // 2], F32,
                                kind="ExternalOutput").ap()
        dbg_gd = nc.dram_tensor("dbg_gd", [C + 128, DS], F16,
                                kind="ExternalOutput").ap()

    with tile.TileContext(nc) as tc:
        with tc.tile_pool(name="dram", bufs=1, space="DRAM") as dram:
            xd_u = dram.tile([C, DS], F16, name="xd_u")
            xd_d = dram.tile([C, DS], F16, name="xd_d")
            gd_u = dram.tile([C + 128, DS], F16, name="gd_u")
            gd_d = dram.tile([C + 128, DS], F16, name="gd_d")
            up_loc = dram.tile([R, DS // 2], F32, name="up_loc")
            up_all = dram.tile([NC * R, DS // 2], F32, addr_space="Shared",
                               name="up_all")
            dn_loc = dram.tile([DS, R // 2], F32, name="dn_loc")
            dn_all = dram.tile([D, R // 2], F32, addr_space="Shared",
                               name="dn_all")

            with tc.tile_pool(name="pre", bufs=1) as pre:
                lh1 = pre.tile([128, 64 * 128], F16, name="lh1")
                nc.sync.dma_start(lh1[:], lh1_in[:])
                lh2 = pre.tile([128, 64 * 128], F16, name="lh2")
                nc.sync.dma_start(lh2[:], lh2_in[:])
                gidx = pre.tile([128, gidx_cols], I16, name="gidx")
                nc.sync.dma_start(gidx[:], gidx_in[:])
                ident = pre.tile([128, 128], F16, name="ident")
                make_identity(nc, ident[:])
                zrow = pre.tile([128, DS], F16, name="zrow")
                nc.gpsimd.memset(zrow[:], 0.0)
                nc.sync.dma_start(gd_u[ZROW:ZROW + 128, :], zrow[:])
                nc.sync.dma_start(gd_d[ZROW:ZROW + 128, :], zrow[:])

                upsb = pre.tile([128, 8, DS], F16, name="upsb")
                dnr = pre.tile([128, 8, DS], F16, name="dnr")

                def fwht_pass(w_in, xd, gd, sfx):
                    """G -> gd rows g = t*128 + h*64 + iA for c = iA*128+2t+h.

                    xd row order: m = iB*64 + iA' so stage-2 tile t reads
                    xd[128t:128(t+1)] contiguously with partition
                    q = h*64 + iA' (iB = 2t + h)."""
                    xdw = xd.rearrange("(ib ia) d -> ib ia d", ia=64)
                    with (
                        tc.tile_pool(name=f"s1{sfx}", bufs=4) as s1p,
                        tc.tile_pool(name=f"ps1{sfx}", bufs=4,
                                     space="PSUM") as ps1,
                    ):
                        for ia in range(64):
                            rw = s1p.tile([128, DS], F16, tag="rw")
                            nc.sync.dma_start(
                                rw[:], w_in[128 * ia:128 * (ia + 1), :])
                            p1 = ps1.tile([128, DS], F32, tag="p1")
                            nc.tensor.matmul(
                                p1[:], lh1[:, 128 * ia:128 * (ia + 1)], rw[:],
                                start=True, stop=True)
                            xe = s1p.tile([128, DS], F16, tag="xe")
                            if ia % 2 == 0:
                                nc.scalar.activation(xe[:], p1[:], AF.Copy)
                            else:
                                nc.vector.tensor_copy(xe[:], p1[:])
                            nc.sync.dma_start(
                                xdw[:, ia, :], xe[:])
                        for t in range(64):
                            r2 = s1p.tile([128, DS], F16, tag="r2")
                            nc.sync.dma_start(
                                r2[:], xd[128 * t:128 * (t + 1), :])
                            p2 = ps1.tile([128, DS], F32, tag="p2")
                            nc.tensor.matmul(
                                p2[:], lh2[:, 128 * t:128 * (t + 1)], r2[:],
                                start=True, stop=True)
                            ge = s1p.tile([128, DS], F16, tag="ge")
                            if t % 2 == 0:
                                nc.scalar.activation(ge[:], p2[:], AF.Copy)
                            else:
                                nc.vector.tensor_copy(ge[:], p2[:])
                            nc.sync.dma_start(
                                gd[128 * t:128 * (t + 1), :], ge[:])

                def gather_pass(gd, res, sfx):
                    """res[:, j, :] = sum_s G[clist[128j+p][s], :].

                    Each dma_gather is capped at 8 slots = 1024 indices:
                    the SWDGE descriptor carveout holds 1024 descriptors."""
                    with tc.tile_pool(name=f"g{sfx}", bufs=1) as gp:
                        off = 0
                        for j in range(8):
                            sj = pads[j]
                            gt = gp.tile([128, smax, DS], F16, tag="gt",
                                         bufs=2)
                            done = 0
                            while done < sj:
                                k = min(8, sj - done)
                                nidx = 128 * k
                                nc.gpsimd.dma_gather(
                                    gt[:, done:done + k, :], gd[:],
                                    gidx[:, off:off + nidx // 16],
                                    nidx, nidx, DS)
                                off += nidx // 16
                                done += k
                            _tree_sum(nc, gt, sj, res[:, j, :])

                # ---- up weight ----
                fwht_pass(wup_in, xd_u, gd_u, "u")
                gather_pass(gd_u, upsb, "u")
                nc.sync.dma_start(
                    up_loc.bitcast(F16).rearrange("(j p) d -> p j d", p=128),
                    upsb[:])
                nc.gpsimd.collective_compute(
                    "AllGather", mybir.AluOpType.bypass,
                    replica_groups=[list(range(NC))],
                    ins=[up_loc.opt()], outs=[up_all.opt()])

                # ---- down weight ----
                fwht_pass(wdn_in, xd_d, gd_d, "d")
                gather_pass(gd_d, dnr, "d")
                # transpose dnr [1024 r, 512 d] -> dnt [512 d, 1024 r]
                with (
                    tc.tile_pool(name="dnt", bufs=1) as dntp,
                    tc.tile_pool(name="pst", bufs=4, space="PSUM") as pst,
                ):
                    dnt = dntp.tile([128, 4, R], F16, name="dnt")
                    for a in range(4):
                        for j in range(8):
                            tp = pst.tile([128, 128], F32, tag="tp")
                            nc.tensor.matmul(
                                tp[:], dnr[:, j, 128 * a:128 * (a + 1)],
                                ident[:], start=True, stop=True)
                            if (a * 8 + j) % 2 == 0:
                                nc.scalar.activation(
                                    dnt[:, a, 128 * j:128 * (j + 1)], tp[:],
                                    AF.Copy)
                            else:
                                nc.vector.tensor_copy(
                                    dnt[:, a, 128 * j:128 * (j + 1)], tp[:])
                    nc.sync.dma_start(
                        dn_loc.bitcast(F16).rearrange("(a p) r -> p a r",
                                                      p=128),
                        dnt[:])
                nc.gpsimd.collective_compute(
                    "AllGather", mybir.AluOpType.bypass,
                    replica_groups=[list(range(NC))],
                    ins=[dn_loc.opt()], outs=[dn_all.opt()])

            if debug:
                with tc.tile_pool(name="dbg", bufs=2) as dbp:
                    for blk in range(NC * R // 128):
                        t_ = dbp.tile([128, DS // 2], F32, tag="du")
                        nc.sync.dma_start(
                            t_[:], up_all[128 * blk:128 * (blk + 1), :])
                        nc.sync.dma_start(
                            dbg_up[128 * blk:128 * (blk + 1), :], t_[:])
                    for blk in range(D // 128):
                        t_ = dbp.tile([128, R // 2], F32, tag="dd")
                        nc.sync.dma_start(
                            t_[:], dn_all[128 * blk:128 * (blk + 1), :])
                        nc.sync.dma_start(
                            dbg_dn[128 * blk:128 * (blk + 1), :], t_[:])
                    for blk in range((C + 128) // 128):
                        t_ = dbp.tile([128, DS], F16, tag="dg")
                        nc.sync.dma_start(
                            t_[:], gd_u[128 * blk:128 * (blk + 1), :])
                        nc.sync.dma_start(
                            dbg_gd[128 * blk:128 * (blk + 1), :], t_[:])

            # ================= main phase =================
            with tc.tile_pool(name="mainw", bufs=1) as mw:
                upg = mw.tile([128, 8, D], F16, name="upg")
                for m in range(NC):
                    nc.sync.dma_start(
                        upg[:, :, DS * m:DS * (m + 1)],
                        up_all[R * m:R * (m + 1), :].bitcast(F16)
                        .rearrange("(rh p) d -> p rh d", p=128))
                dnsb = mw.tile([128, 32, R], F16, name="dnsb")
                for m in range(NC):
                    nc.sync.dma_start(
                        dnsb[:, 4 * m:4 * (m + 1), :],
                        dn_all[DS * m:DS * (m + 1), :].bitcast(F16)
                        .rearrange("(a p) r -> p a r", p=128))

                with (
                    tc.tile_pool(name="mt", bufs=1) as mt,
                    tc.tile_pool(name="psh", bufs=4, space="PSUM") as psh,
                    tc.tile_pool(name="pso", bufs=3, space="PSUM") as pso,
                ):
                    xtv = xt_in.rearrange("(rh p) t -> p rh t", p=128)
                    for b in range(TS // TQ):
                        xq = mt.tile([128, 8, TQ], F16, tag="xq", bufs=2)
                        nc.sync.dma_start(
                            xq[:], xtv[:, :, TQ * b:TQ * (b + 1)])
                        hb = mt.tile([128, 32, TQ], F16, tag="hb", bufs=1)
                        for dt in range(32):
                            ph = psh.tile([128, TQ], F32, tag="ph")
                            for rh in range(8):
                                nc.tensor.matmul(
                                    ph[:],
                                    upg[:, rh, 128 * dt:128 * (dt + 1)],
                                    xq[:, rh, :],
                                    start=(rh == 0), stop=(rh == 7))
                            nc.scalar.activation(hb[:, dt, :], ph[:], AF.Gelu)
                        for tt in range(TQ // 128):
                            for rr in range(2):
                                po = pso.tile([128, 512], F32, tag="po")
                                for dk in range(32):
                                    nc.tensor.matmul(
                                        po[:],
                                        hb[:, dk, 128 * tt:128 * (tt + 1)],
                                        dnsb[:, dk, 512 * rr:512 * (rr + 1)],
                                        start=(dk == 0), stop=(dk == 31))
                                ot = mt.tile([128, 512], F32, tag="ot",
                                             bufs=3)
                                nc.vector.tensor_copy(ot[:], po[:])
                                nc.sync.dma_start(
                                    out_ext[TQ * b + 128 * tt:
                                            TQ * b + 128 * (tt + 1),
                                            512 * rr:512 * (rr + 1)],
                                    ot[:])

    nc.compile()
    return nc


def _get_nc(pads):
    key = (tuple(pads), bool(os.environ.get("KERNEL_DEBUG")))
    if key not in _NC_CACHE:
        _NC_CACHE[key] = _build(key[0])
    return _NC_CACHE[key]


def kernel(x, random_sign, proj_indices, proj_values, w_up, w_down):
    global last_exec_time_ns
    x = np.ascontiguousarray(np.asarray(x, dtype=np.float32))
    sign = np.asarray(random_sign, dtype=np.float32)
    pi = np.asarray(proj_indices)
    pv = np.asarray(proj_values, dtype=np.float32)
    w_up = np.asarray(w_up, dtype=np.float32)
    w_down = np.asarray(w_down, dtype=np.float32)

    # ---- host marshalling ----
    rows = pi[0].astype(np.int64)
    cols = pi[1].astype(np.int64)
    # v per hadamard column (one nonzero per column of the projection)
    v_col = np.zeros(C, dtype=np.float64)
    np.add.at(v_col, cols, pv.astype(np.float64))
    # c-lists per output row
    clist = [[] for _ in range(R)]
    order = np.argsort(rows, kind="stable")
    for j in order:
        clist[rows[j]].append(int(cols[j]))

    pads = []
    for blk in range(8):
        m = max(len(clist[r]) for r in range(128 * blk, 128 * (blk + 1)))
        pads.append(max(2, m))

    # gd row of hadamard column c = iA*128 + iB (see fwht_pass docstring)
    def gd_row(c):
        ia, ib = divmod(c, 128)
        return (ib // 2) * 128 + (ib % 2) * 64 + ia

    # gather indices: batch j, token i = s*128 + rr -> row 128j+rr slot s
    gidx_parts = []
    for blk in range(8):
        sj = pads[blk]
        idx = np.full(sj * 128, ZROW, dtype=np.int16)
        for rr in range(128):
            cl = clist[128 * blk + rr]
            for s, c in enumerate(cl):
                idx[s * 128 + rr] = gd_row(c)
        w16 = idx.reshape(-1, 16).T  # [16, nidx/16]
        gidx_parts.append(np.tile(w16, (8, 1)))
    gidx_host = np.ascontiguousarray(np.concatenate(gidx_parts, axis=1))

    # lh1[iB', iA'*128 + iB] = H128[iB', iB] * sign[iA'*128 + iB'] * SCALE
    H128 = _hadamard(128)
    H64 = _hadamard(64)
    sgn = sign.astype(np.float64).reshape(64, 128)  # [iA', iB']
    lh1 = (sgn[:, :, None] * H128[None, :, :] * SCALE)  # [iA', iB', iB]
    lh1_host = np.ascontiguousarray(
        lh1.transpose(1, 0, 2).reshape(128, 64 * 128).astype(np.float16))

    # lh2[(h,iA'), t*128 + (h',iA)] = delta(h,h') H64[iA',iA] v[iA*128+2t+h]
    vmat = v_col.reshape(64, 64, 2)  # [iA, t, h]
    lh2 = np.zeros((2, 64, 64, 2, 64), dtype=np.float64)  # [h, iA', t, h', iA]
    for h in range(2):
        lh2[h, :, :, h, :] = H64[:, None, :] * vmat[None, :, :, h].transpose(
            0, 2, 1)
    lh2_host = np.ascontiguousarray(
        lh2.reshape(128, 64, 128).reshape(128, 64 * 128).astype(np.float16))

    xT = np.ascontiguousarray(x.T).astype(np.float16)
    wupT = np.ascontiguousarray(w_up.T.astype(np.float16))
    wdn16 = w_down.astype(np.float16)

    in_maps = []
    for k in range(NC):
        in_maps.append({
            "xt_in": np.ascontiguousarray(xT[:, TS * k:TS * (k + 1)]),
            "wup_in": np.ascontiguousarray(wupT[:, DS * k:DS * (k + 1)]),
            "wdn_in": np.ascontiguousarray(wdn16[:, DS * k:DS * (k + 1)]),
            "lh1_in": lh1_host,
            "lh2_in": lh2_host,
            "gidx_in": gidx_host,
        })

    trace = bool(os.environ.get("KERNEL_TRACE"))
    if trace:
        _register_ntff_hook()
    nc = _get_nc(pads)
    res = run_bass_kernel_spmd(nc, in_maps, core_ids=list(range(NC)), trace=trace)
    last_exec_time_ns = res.exec_time_ns
    if os.environ.get("KERNEL_DEBUG"):
        global last_debug
        last_debug = res.results
    return np.concatenate([res.results[k]["out"] for k in range(NC)], axis=0)
